# revision 29
# baseline (speedup 1.0000x reference)
"""Trainium2 Bass kernel: batched 4-point DLT homography (closed-form solve).

Contract: kernel(pts_1_tile, pred_h4p_tile) -> [B, 3, 3] float32, with
B = 524288 split across 8 NeuronCores (batch-parallel, no communication).

Math (per batch element, points p=0..3 with src (x_p,y_p), dst (X_p,Y_p)):
the DLT system rows are
    x h0 + y h1 + h2 = X (1 + x h6 + y h7)
    x h3 + y h4 + h5 = Y (1 + x h6 + y h7)
Eliminating (h0,h1,h2) from the four X-equations via the left null vector n
of M = [(x_p, y_p, 1)] gives one linear equation in (h6,h7); same for the
Y-equations. Solve the 2x2, back out the rest in closed form.

Layout strategy: the host pre-transposes inputs into PLANAR fp16 feature
planes (x0..x3,y0..y3 and X0..X3,Y0..Y3 with X=x+pred precomputed) so the
device does zero deinterleaving and DMAs half the bytes. Each per-element
scalar is a [128, 512] plane; planes are placed at hand-chosen offsets so
nearly every step fuses into a single multi-plane DVE instruction with
affine access patterns. Outputs leave as 8 planar fp16 planes (h0..h7);
the host casts/interleaves into [B,3,3] fp32 and appends the ones.
"""
import sys

for _p in ("/opt/trn_rl_repo", "/root/.axon_site/_ro/trn_rl_repo"):
    if _p not in sys.path:
        sys.path.append(_p)

import numpy as np

import concourse.bass as bass
import concourse.mybir as mybir
from concourse import bacc
from concourse.tile import TileContext
from concourse.bass_utils import run_bass_kernel_spmd

N_CORES = 8
B_TOTAL = 524288
PER_CORE = B_TOTAL // N_CORES  # 65536
P = 128
F = PER_CORE // P  # 512
FP32 = mybir.dt.float32
FP16 = mybir.dt.float16

ADD = mybir.AluOpType.add
SUB = mybir.AluOpType.subtract
MUL = mybir.AluOpType.mult

# fp16 slab plane offsets ---------------------------------------------------
_X = 0        # x0 x1 x2 x3 y0 y1 y2 y3
_U = 8        # X0 X1 X2 X3 Y0 Y1 Y2 Y3
_DD = 16      # dx1 dx2 dx3 dy1 dy2 dy3
_PA = 22      # dx2*dy3, dx3*dy1, dx1*dy2
_PB = 25      # dx3*dy2, dx1*dy3, dx2*dy1
_NS = 28      # n0 n1 n2 n3
_T0 = 32
_Z = 33       # z0..z3 (n*X), z'0..z'3 (n*Y)
_ZX = 41      # zx(4) zy(4) zx'(4) zy'(4)
_S1 = 57      # pair sums of z.._ZX (12)
_DOT = 69     # aX aY bX cX bY cY
_PC = 75      # bX*cY, cX*aY, bY*aX
_PD = 78      # bY*cX, cY*aX, bX*aY
_DT = 81      # det h6n h7n
_M12 = 84     # x0h6 x1h6 x2h6 y0h7 y1h7 y2h7
_W = 90       # w0 w1 w2  (1 + x h6 + y h7)
_XW = 93      # XW0 XW1 XW2 YW0 YW1 YW2
_PQ = 99      # P1 P2 Q1 Q2
_PE = 103     # dy2*P1 dy2*Q1 dx1*P2 dx1*Q2
_PF = 107     # dy1*P2 dy1*Q2 dx2*P1 dx2*Q1
_HN = 111     # h0n h3n h1n h4n
_RDET = 115   # 1/det (fp16)
_RD = 116     # 1/n3 (fp16)
_OUT = 117    # h0 h3 h1 h4 h2 h5 h6 h7
_EE = 125     # x0h0 x0h3 y0h1 y0h4
_S1P = 129    # XW0-x0h0, YW0-x0h3
_SP = 131     # m1+m2
NP = 134

# fp32 slab planes: det32, rdet32, n332, rd32
N32 = 4

# device plane order -> H flat index (h0 h3 h1 h4 h2 h5 h6 h7)
_PLANE2H = [0, 3, 1, 4, 2, 5, 6, 7]
# input feature order: interleaved (x0,y0,x1,y1,..) -> planar (x0..x3,y0..y3)
_PERM = [0, 2, 4, 6, 1, 3, 5, 7]

_EYE = np.ascontiguousarray(np.eye(128, dtype=np.float16))


def _build():
    nc = bacc.Bacc(None, target_bir_lowering=False, debug=True)
    xy = nc.dram_tensor("xy", [8, PER_CORE], FP16, kind="ExternalInput")
    uu = nc.dram_tensor("uu", [8, PER_CORE], FP16, kind="ExternalInput")
    eye = nc.dram_tensor("eye", [P, P], FP16, kind="ExternalInput")
    out = nc.dram_tensor("out", [8, PER_CORE], FP16, kind="ExternalOutput")

    with TileContext(nc) as tc:
        with tc.tile_pool(name="s", bufs=1) as pool, tc.tile_pool(
            name="ps", bufs=1, space="PSUM"
        ) as ppool:
            slab = pool.tile([P, NP * F], FP16, tag="slab")
            s32 = pool.tile([P, N32 * F], FP32, tag="s32")
            eyet = pool.tile([P, P], FP16, tag="eye")
            # 6 dot accumulators + 1 warmup scratch (7 of 8 PSUM banks)
            psum = [
                ppool.tile([P, F], FP32, name=f"psum{i}", tag=f"d{i}")
                for i in range(7)
            ]

            def R(o, n):
                return slab[:, o * F : (o + n) * F]

            def R32(o, n):
                return s32[:, o * F : (o + n) * F]

            def ab(o, n, a, b):
                return R(o, n).rearrange("p (a b f) -> p a b f", a=a, b=b)

            def pl(o, n):
                return R(o, n).rearrange("p (b f) -> p b f", f=F)

            def bc1(o, k):
                # one plane broadcast over k
                return R(o, 1).unsqueeze(1).broadcast_to((P, k, F))

            def sv(base, d1, n1, d2=None, n2=None):
                # strided plane view [P, n1(, n2), F]; d1/d2 are plane
                # strides and may be negative (HW APs are signed)
                if d2 is None:
                    view = pl(0, n1).copy()
                    view.ap[1] = (d1 * F, n1)
                else:
                    view = ab(0, n1 * n2, n1, n2).copy()
                    view.ap[1] = (d1 * F, n1)
                    view.ap[2] = (d2 * F, n2)
                view.offset = base * F
                return view

            v = nc.vector
            g = nc.gpsimd
            s = nc.scalar

            # input DMAs on the scalar (Act) HW-DGE queue: it is ready early
            # and FIFO order streams x, then y, then u at full bandwidth, so
            # the first diff instruction can start right after the x planes.
            xyr = xy.rearrange("k (p f) -> p k f", p=P)
            s.dma_start(out=pl(_X, 4), in_=xyr[:, 0:4, :])
            s.dma_start(out=pl(_X + 4, 4), in_=xyr[:, 4:8, :])
            s.dma_start(out=pl(_U, 8), in_=uu.rearrange("k (p f) -> p k f", p=P))
            nc.sync.dma_start(out=eyet[:, :], in_=eye[:, :])

            # PE warmup: keep the tensor engine busy from DMA-arrival until
            # the real accumulations so HAM unthrottles it (1.2 -> 2.4 GHz).
            t = nc.tensor
            t.ldweights(weights=eyet[:, :])
            for k in range(26):
                t.matmul(
                    out=psum[6][:, :], lhsT=eyet[:, :], rhs=R(_X + (k % 8), 1),
                    start=True, stop=True,
                )

            xv = ab(_X, 8, 2, 4)  # [p, xy, point, f]

            # diffs, split so the x half starts before the y planes arrive
            v.tensor_tensor(  # dx_i = x_i - x0
                out=pl(_DD, 3),
                in0=pl(_X + 1, 3),
                in1=bc1(_X, 3),
                op=SUB,
            )
            v.tensor_tensor(  # dy_i = y_i - y0
                out=pl(_DD + 3, 3),
                in0=pl(_X + 5, 3),
                in1=bc1(_X + 4, 3),
                op=SUB,
            )
            DX1, DX2, DX3, DY1, DY2, DY3 = range(_DD, _DD + 6)

            # cross products for the null vector n: three dx-shared pair
            # instructions (pa_k | pb_j), strided/negative plane views
            v.tensor_tensor(  # dx2*(dy3,dy1) -> pa0, pb2
                out=sv(_PA, 5, 2), in0=bc1(DX2, 2), in1=sv(DY3, -2, 2), op=MUL)
            v.tensor_tensor(  # dx3*(dy1,dy2) -> pa1, pb0
                out=sv(_PA + 1, 2, 2), in0=bc1(DX3, 2), in1=sv(DY1, 1, 2), op=MUL)
            v.tensor_tensor(  # dx1*(dy2,dy3) -> pa2, pb1
                out=sv(_PA + 2, 2, 2), in0=bc1(DX1, 2), in1=sv(DY2, 1, 2), op=MUL)
            v.tensor_tensor(out=R(_NS + 1, 3), in0=R(_PA, 3), in1=R(_PB, 3), op=SUB)
            v.tensor_tensor(out=R(_T0, 1), in0=R(_NS + 1, 1), in1=R(_NS + 2, 1), op=ADD)
            v.scalar_tensor_tensor(
                out=R(_NS, 1), in0=R(_T0, 1), scalar=-1.0, in1=R(_NS + 3, 1),
                op0=MUL, op1=SUB,
            )  # n0 = -(n1+n2)-n3

            # 1/n3 chain part 1 (scalar engine casts to fp32)
            s.copy(out=R32(2, 1), in_=R(_NS + 3, 1))

            # z = n * X (4 planes) and z' = n * Y (4 planes), one instr
            v.tensor_tensor(
                out=ab(_Z, 8, 2, 4),
                in0=pl(_NS, 4).unsqueeze(1).broadcast_to((P, 2, 4, F)),
                in1=ab(_U, 8, 2, 4),
                op=MUL,
            )
            def dot_mm(bank, base):
                # psum[bank] = sum of 4 planes via identity-weight matmuls
                for q in range(4):
                    t.matmul(
                        out=psum[bank][:, :], lhsT=eyet[:, :],
                        rhs=R(base + q, 1), start=(q == 0), stop=(q == 3),
                    )

            # aX = sum z, aY = sum z' on the tensor engine (runs in the
            # shadow of the big V product instructions below)
            dot_mm(0, _Z)
            dot_mm(1, _Z + 4)

            # zx=z*x, zy=z*y (8 planes)
            v.tensor_tensor(
                out=ab(_ZX, 8, 2, 4),
                in0=pl(_Z, 4).unsqueeze(1).broadcast_to((P, 2, 4, F)),
                in1=ab(_X, 8, 2, 4),
                op=MUL,
            )
            dot_mm(2, _ZX)      # bX
            dot_mm(3, _ZX + 4)  # cX
            # zx'=z'*x, zy'=z'*y (8 planes)
            v.tensor_tensor(
                out=ab(_ZX + 8, 8, 2, 4),
                in0=pl(_Z + 4, 4).unsqueeze(1).broadcast_to((P, 2, 4, F)),
                in1=ab(_X, 8, 2, 4),
                op=MUL,
            )
            dot_mm(4, _ZX + 8)   # bY
            dot_mm(5, _ZX + 12)  # cY

            # 1/n3 chain part 2 (placed here so the V op never waits on scalar)
            v.reciprocal_approx_fast(out=R32(3, 1), in_=R32(2, 1))
            s.copy(out=R(_RD, 1), in_=R32(3, 1))

            # dots PSUM(fp32) -> SBUF(fp16) on the scalar engine
            AX, AY, BX, CX, BY, CY = range(_DOT, _DOT + 6)
            for k in range(6):
                s.copy(out=R(_DOT + k, 1), in_=psum[k][:, :])
            # 2x2 cross terms: three shared-factor pair instructions
            v.tensor_tensor(  # bX*(aY,cY) -> pd2, pc0
                out=sv(_PD + 2, -5, 2), in0=bc1(BX, 2), in1=sv(AY, 4, 2), op=MUL)
            v.tensor_tensor(  # cX*(aY,bY) -> pc1, pd0
                out=sv(_PC + 1, 2, 2), in0=bc1(CX, 2), in1=sv(AY, 3, 2), op=MUL)
            v.tensor_tensor(  # aX*(bY,cY) -> pc2, pd1
                out=sv(_PC + 2, 2, 2), in0=bc1(AX, 2), in1=sv(BY, 1, 2), op=MUL)
            v.tensor_tensor(out=R(_DT, 3), in0=R(_PC, 3), in1=R(_PD, 3), op=SUB)

            # 1/det chain starts on scalar; the n-products below hide it
            s.copy(out=R32(0, 1), in_=R(_DT, 1))

            # mn = x*h6n, y*h7n for p=0..2 (defers the rdet multiply so the
            # reciprocal chain runs on scalar in parallel with these V ops)
            v.tensor_tensor(
                out=ab(_M12, 6, 2, 3),
                in0=xv[:, :, 0:3, :],
                in1=pl(_DT + 1, 2).unsqueeze(2).broadcast_to((P, 2, 3, F)),
                op=MUL,
            )
            v.reciprocal_approx_fast(out=R32(1, 1), in_=R32(0, 1))
            s.copy(out=R(_RDET, 1), in_=R32(1, 1))
            v.tensor_tensor(  # spn = x h6n + y h7n
                out=R(_SP, 3), in0=R(_M12, 3), in1=R(_M12 + 3, 3), op=ADD,
            )
            v.tensor_tensor(  # sp = spn * rdet
                out=pl(_W, 3), in0=pl(_SP, 3), in1=bc1(_RDET, 3), op=MUL,
            )
            v.tensor_scalar_add(out=R(_W, 3), in0=R(_W, 3), scalar1=1.0)
            # XW = w*X, YW = w*Y (p=0..2)
            v.tensor_tensor(
                out=ab(_XW, 6, 2, 3),
                in0=pl(_W, 3).unsqueeze(1).broadcast_to((P, 2, 3, F)),
                in1=ab(_U, 8, 2, 4)[:, :, 0:3, :],
                op=MUL,
            )
            # h6 h7 -> out planes 6,7 (off the critical path)
            v.tensor_tensor(
                out=pl(_OUT + 6, 2),
                in0=pl(_DT + 1, 2),
                in1=bc1(_RDET, 2),
                op=MUL,
            )
            nc.sync.dma_start(
                out=out[6:8, :].rearrange("k (p f) -> p k f", p=P),
                in_=pl(_OUT + 6, 2),
            )
            # P1 P2 Q1 Q2
            v.tensor_tensor(
                out=ab(_PQ, 4, 2, 2),
                in0=ab(_XW, 6, 2, 3)[:, :, 1:3, :],
                in1=ab(_XW, 6, 2, 3)[:, :, 0:1, :].broadcast_to((P, 2, 2, F)),
                op=SUB,
            )
            # pe = (dy2*P1, dy2*Q1, dx1*P2, dx1*Q2), pf likewise: one fused
            # instruction each via signed-stride views
            v.tensor_tensor(
                out=ab(_PE, 4, 2, 2),
                in0=sv(_PQ, 1, 2, 2, 2),       # P1 Q1 / P2 Q2
                in1=sv(DY2, DX1 - DY2, 2, 0, 2),
                op=MUL,
            )
            v.tensor_tensor(
                out=ab(_PF, 4, 2, 2),
                in0=sv(_PQ + 1, -1, 2, 2, 2),  # P2 Q2 / P1 Q1
                in1=sv(DY1, DX2 - DY1, 2, 0, 2),
                op=MUL,
            )
            v.tensor_tensor(out=R(_HN, 4), in0=R(_PE, 4), in1=R(_PF, 4), op=SUB)
            # h0 h3 h1 h4 -> out planes 0..3
            v.tensor_tensor(out=pl(_OUT, 4), in0=pl(_HN, 4), in1=bc1(_RD, 4), op=MUL)
            nc.sync.dma_start(
                out=out[0:4, :].rearrange("k (p f) -> p k f", p=P),
                in_=pl(_OUT, 4),
            )

            # h2 = XW0 - x0 h0 - y0 h1 ; h5 = YW0 - x0 h3 - y0 h4
            v.tensor_tensor(
                out=ab(_EE, 4, 2, 2),
                in0=xv[:, :, 0:1, :].broadcast_to((P, 2, 2, F)),
                in1=ab(_OUT, 4, 2, 2),
                op=MUL,
            )  # (x0h0, x0h3, y0h1, y0h4)
            v.tensor_tensor(
                out=pl(_S1P, 2),
                in0=ab(_XW, 6, 2, 3)[:, :, 0, :],
                in1=pl(_EE, 2),
                op=SUB,
            )
            v.tensor_tensor(
                out=pl(_OUT + 4, 2), in0=pl(_S1P, 2), in1=pl(_EE + 2, 2), op=SUB
            )
            s.dma_start(
                out=out[4:6, :].rearrange("k (p f) -> p k f", p=P),
                in_=pl(_OUT + 4, 2),
            )
    nc.finalize()
    return nc


_NC_CACHE = {}


def _get_nc():
    if "nc" not in _NC_CACHE:
        _NC_CACHE["nc"] = _build()
    return _NC_CACHE["nc"]


def kernel(pts_1_tile, pred_h4p_tile, _trace=False):
    pts = np.asarray(pts_1_tile, dtype=np.float32).reshape(B_TOTAL, 8)
    prd = np.asarray(pred_h4p_tile, dtype=np.float32).reshape(B_TOTAL, 8)
    u = pts + prd
    nc = _get_nc()
    in_maps = []
    for c in range(N_CORES):
        lo, hi = c * PER_CORE, (c + 1) * PER_CORE
        xy = np.ascontiguousarray(pts[lo:hi, _PERM].T.astype(np.float16))
        uu = np.ascontiguousarray(u[lo:hi, _PERM].T.astype(np.float16))
        in_maps.append({"xy": xy, "uu": uu, "eye": _EYE})
    res = run_bass_kernel_spmd(nc, in_maps, list(range(N_CORES)), trace=_trace)
    arr = np.stack([res.results[i]["out"] for i in range(N_CORES)], axis=0)
    H9 = np.empty((N_CORES, PER_CORE, 9), np.float32)
    for k, m in enumerate(_PLANE2H):
        H9[:, :, m] = arr[:, k, :]
    H9[:, :, 8] = 1.0
    H = H9.reshape(B_TOTAL, 3, 3)
    if _trace:
        return H, res
    return H


# revision 30
# speedup vs baseline: 1.0051x; 1.0051x over previous
"""Trainium2 Bass kernel: batched 4-point DLT homography (closed-form solve).

Contract: kernel(pts_1_tile, pred_h4p_tile) -> [B, 3, 3] float32, with
B = 524288 split across 8 NeuronCores (batch-parallel, no communication).

Math (per batch element, points p=0..3 with src (x_p,y_p), dst (X_p,Y_p)):
the DLT system rows are
    x h0 + y h1 + h2 = X (1 + x h6 + y h7)
    x h3 + y h4 + h5 = Y (1 + x h6 + y h7)
Eliminating (h0,h1,h2) from the four X-equations via the left null vector n
of M = [(x_p, y_p, 1)] gives one linear equation in (h6,h7); same for the
Y-equations. Solve the 2x2, back out the rest in closed form.

Layout strategy: the host pre-transposes inputs into PLANAR fp16 feature
planes (x0..x3,y0..y3 and X0..X3,Y0..Y3 with X=x+pred precomputed) so the
device does zero deinterleaving and DMAs half the bytes. Each per-element
scalar is a [128, 512] plane; planes are placed at hand-chosen offsets so
nearly every step fuses into a single multi-plane DVE instruction with
affine access patterns. Outputs leave as 8 planar fp16 planes (h0..h7);
the host casts/interleaves into [B,3,3] fp32 and appends the ones.
"""
import sys

for _p in ("/opt/trn_rl_repo", "/root/.axon_site/_ro/trn_rl_repo"):
    if _p not in sys.path:
        sys.path.append(_p)

import numpy as np

import concourse.bass as bass
import concourse.mybir as mybir
from concourse import bacc
from concourse.tile import TileContext
from concourse.bass_utils import run_bass_kernel_spmd

N_CORES = 8
B_TOTAL = 524288
PER_CORE = B_TOTAL // N_CORES  # 65536
P = 128
F = PER_CORE // P  # 512
FP32 = mybir.dt.float32
FP16 = mybir.dt.float16

ADD = mybir.AluOpType.add
SUB = mybir.AluOpType.subtract
MUL = mybir.AluOpType.mult

# fp16 slab plane offsets ---------------------------------------------------
_X = 0        # x0 x1 x2 x3 y0 y1 y2 y3
_U = 8        # X0 X1 X2 X3 Y0 Y1 Y2 Y3
_DD = 16      # dx1 dx2 dx3 dy1 dy2 dy3
_PA = 22      # dx2*dy3, dx3*dy1, dx1*dy2
_PB = 25      # dx3*dy2, dx1*dy3, dx2*dy1
_NS = 28      # n0 n1 n2 n3
_T0 = 32
_Z = 33       # z0..z3 (n*X), z'0..z'3 (n*Y)
_ZX = 41      # zx(4) zy(4) zx'(4) zy'(4)
_S1 = 57      # pair sums of z.._ZX (12)
_DOT = 69     # aX aY bX cX bY cY
_PC = 75      # bX*cY, cX*aY, bY*aX
_PD = 78      # bY*cX, cY*aX, bX*aY
_DT = 81      # det h6n h7n
_M12 = 84     # x0h6 x1h6 x2h6 y0h7 y1h7 y2h7
_W = 90       # w0 w1 w2  (1 + x h6 + y h7)
_XW = 93      # XW0 XW1 XW2 YW0 YW1 YW2
_PQ = 99      # P1 P2 Q1 Q2
_PE = 103     # dy2*P1 dy2*Q1 dx1*P2 dx1*Q2
_PF = 107     # dy1*P2 dy1*Q2 dx2*P1 dx2*Q1
_HN = 111     # h0n h3n h1n h4n
_RDET = 115   # 1/det (fp16)
_RD = 116     # 1/n3 (fp16)
_OUT = 117    # h0 h3 h1 h4 h2 h5 h6 h7
_EE = 125     # x0h0 x0h3 y0h1 y0h4
_S1P = 129    # XW0-x0h0, YW0-x0h3
_SP = 131     # m1+m2
NP = 134

# fp32 slab planes: det32, rdet32, n332, rd32
N32 = 4

# device plane order -> H flat index (h0 h3 h1 h4 h2 h5 h6 h7)
_PLANE2H = [0, 3, 1, 4, 2, 5, 6, 7]
# input feature order: interleaved (x0,y0,x1,y1,..) -> planar (x0..x3,y0..y3)
_PERM = [0, 2, 4, 6, 1, 3, 5, 7]

_EYE = np.ascontiguousarray(np.eye(128, dtype=np.float16))


def _build():
    nc = bacc.Bacc(None, target_bir_lowering=False, debug=True)
    xy = nc.dram_tensor("xy", [8, PER_CORE], FP16, kind="ExternalInput")
    uu = nc.dram_tensor("uu", [8, PER_CORE], FP16, kind="ExternalInput")
    eye = nc.dram_tensor("eye", [P, P], FP16, kind="ExternalInput")
    out = nc.dram_tensor("out", [8, PER_CORE], FP16, kind="ExternalOutput")

    with TileContext(nc) as tc:
        with tc.tile_pool(name="s", bufs=1) as pool, tc.tile_pool(
            name="ps", bufs=1, space="PSUM"
        ) as ppool:
            slab = pool.tile([P, NP * F], FP16, tag="slab")
            s32 = pool.tile([P, N32 * F], FP32, tag="s32")
            eyet = pool.tile([P, P], FP16, tag="eye")
            # 6 dot accumulators + 1 warmup scratch (7 of 8 PSUM banks)
            psum = [
                ppool.tile([P, F], FP32, name=f"psum{i}", tag=f"d{i}")
                for i in range(7)
            ]

            def R(o, n):
                return slab[:, o * F : (o + n) * F]

            def R32(o, n):
                return s32[:, o * F : (o + n) * F]

            def ab(o, n, a, b):
                return R(o, n).rearrange("p (a b f) -> p a b f", a=a, b=b)

            def pl(o, n):
                return R(o, n).rearrange("p (b f) -> p b f", f=F)

            def bc1(o, k):
                # one plane broadcast over k
                return R(o, 1).unsqueeze(1).broadcast_to((P, k, F))

            def sv(base, d1, n1, d2=None, n2=None):
                # strided plane view [P, n1(, n2), F]; d1/d2 are plane
                # strides and may be negative (HW APs are signed)
                if d2 is None:
                    view = pl(0, n1).copy()
                    view.ap[1] = (d1 * F, n1)
                else:
                    view = ab(0, n1 * n2, n1, n2).copy()
                    view.ap[1] = (d1 * F, n1)
                    view.ap[2] = (d2 * F, n2)
                view.offset = base * F
                return view

            v = nc.vector
            g = nc.gpsimd
            s = nc.scalar

            # input DMAs on the scalar (Act) HW-DGE queue: it is ready early
            # and FIFO order streams x, then y, then u at full bandwidth, so
            # the first diff instruction can start right after the x planes.
            xyr = xy.rearrange("k (p f) -> p k f", p=P)
            s.dma_start(out=pl(_X, 4), in_=xyr[:, 0:4, :])
            s.dma_start(out=pl(_X + 4, 4), in_=xyr[:, 4:8, :])
            s.dma_start(out=pl(_U, 8), in_=uu.rearrange("k (p f) -> p k f", p=P))
            nc.sync.dma_start(out=eyet[:, :], in_=eye[:, :])

            # PE warmup: keep the tensor engine busy from DMA-arrival until
            # the real accumulations so HAM unthrottles it (1.2 -> 2.4 GHz).
            t = nc.tensor
            t.ldweights(weights=eyet[:, :])
            for k in range(26):
                t.matmul(
                    out=psum[6][:, :], lhsT=eyet[:, :], rhs=R(_X + (k % 8), 1),
                    start=True, stop=True,
                )

            xv = ab(_X, 8, 2, 4)  # [p, xy, point, f]

            # diffs, split so the x half starts before the y planes arrive
            v.tensor_tensor(  # dx_i = x_i - x0
                out=pl(_DD, 3),
                in0=pl(_X + 1, 3),
                in1=bc1(_X, 3),
                op=SUB,
            )
            v.tensor_tensor(  # dy_i = y_i - y0
                out=pl(_DD + 3, 3),
                in0=pl(_X + 5, 3),
                in1=bc1(_X + 4, 3),
                op=SUB,
            )
            DX1, DX2, DX3, DY1, DY2, DY3 = range(_DD, _DD + 6)

            # cross products for the null vector n: three dx-shared pair
            # instructions (pa_k | pb_j), strided/negative plane views
            v.tensor_tensor(  # dx2*(dy3,dy1) -> pa0, pb2
                out=sv(_PA, 5, 2), in0=bc1(DX2, 2), in1=sv(DY3, -2, 2), op=MUL)
            v.tensor_tensor(  # dx3*(dy1,dy2) -> pa1, pb0
                out=sv(_PA + 1, 2, 2), in0=bc1(DX3, 2), in1=sv(DY1, 1, 2), op=MUL)
            v.tensor_tensor(  # dx1*(dy2,dy3) -> pa2, pb1
                out=sv(_PA + 2, 2, 2), in0=bc1(DX1, 2), in1=sv(DY2, 1, 2), op=MUL)
            v.tensor_tensor(out=R(_NS + 1, 3), in0=R(_PA, 3), in1=R(_PB, 3), op=SUB)
            v.tensor_tensor(out=R(_T0, 1), in0=R(_NS + 1, 1), in1=R(_NS + 2, 1), op=ADD)
            v.scalar_tensor_tensor(
                out=R(_NS, 1), in0=R(_T0, 1), scalar=-1.0, in1=R(_NS + 3, 1),
                op0=MUL, op1=SUB,
            )  # n0 = -(n1+n2)-n3

            # 1/n3 chain part 1 (scalar engine casts to fp32)
            s.copy(out=R32(2, 1), in_=R(_NS + 3, 1))

            # z = n * X (4 planes) and z' = n * Y (4 planes), one instr
            v.tensor_tensor(
                out=ab(_Z, 8, 2, 4),
                in0=pl(_NS, 4).unsqueeze(1).broadcast_to((P, 2, 4, F)),
                in1=ab(_U, 8, 2, 4),
                op=MUL,
            )
            def dot_mm(bank, base):
                # psum[bank] = sum of 4 planes via identity-weight matmuls
                for q in range(4):
                    t.matmul(
                        out=psum[bank][:, :], lhsT=eyet[:, :],
                        rhs=R(base + q, 1), start=(q == 0), stop=(q == 3),
                    )

            # aX = sum z, aY = sum z' on the tensor engine (runs in the
            # shadow of the big V product instructions below)
            dot_mm(0, _Z)
            dot_mm(1, _Z + 4)

            # zx=z*x, zy=z*y (8 planes)
            v.tensor_tensor(
                out=ab(_ZX, 8, 2, 4),
                in0=pl(_Z, 4).unsqueeze(1).broadcast_to((P, 2, 4, F)),
                in1=ab(_X, 8, 2, 4),
                op=MUL,
            )
            dot_mm(2, _ZX)      # bX
            dot_mm(3, _ZX + 4)  # cX
            # zx'=z'*x, zy'=z'*y (8 planes)
            v.tensor_tensor(
                out=ab(_ZX + 8, 8, 2, 4),
                in0=pl(_Z + 4, 4).unsqueeze(1).broadcast_to((P, 2, 4, F)),
                in1=ab(_X, 8, 2, 4),
                op=MUL,
            )
            dot_mm(4, _ZX + 8)   # bY
            dot_mm(5, _ZX + 12)  # cY

            # 1/n3 chain part 2 (placed here so the V op never waits on scalar)
            v.reciprocal_approx_fast(out=R32(3, 1), in_=R32(2, 1))
            s.copy(out=R(_RD, 1), in_=R32(3, 1))

            # dots PSUM(fp32) -> SBUF(fp16) on the scalar engine
            AX, AY, BX, CX, BY, CY = range(_DOT, _DOT + 6)
            for k in range(6):
                s.copy(out=R(_DOT + k, 1), in_=psum[k][:, :])
            # 2x2 cross terms as singles: they interleave with the PSUM->SBUF
            # copy pipeline better than fused pairs (each pair would gate on
            # the last Y-side dot)
            for k, (a, b) in enumerate(((CX, AY), (BX, AY), (BY, AX))):
                dst = (_PC + 1, _PD + 2, _PC + 2)[k]
                v.tensor_tensor(out=R(dst, 1), in0=R(a, 1), in1=R(b, 1), op=MUL)
            for k, (a, b) in enumerate(((BY, CX), (BX, CY), (CY, AX))):
                dst = (_PD, _PC, _PD + 1)[k]
                v.tensor_tensor(out=R(dst, 1), in0=R(a, 1), in1=R(b, 1), op=MUL)
            v.tensor_tensor(out=R(_DT, 3), in0=R(_PC, 3), in1=R(_PD, 3), op=SUB)

            # 1/det chain starts on scalar; the n-products below hide it
            s.copy(out=R32(0, 1), in_=R(_DT, 1))

            # mn = x*h6n, y*h7n for p=0..2 (defers the rdet multiply so the
            # reciprocal chain runs on scalar in parallel with these V ops)
            v.tensor_tensor(
                out=ab(_M12, 6, 2, 3),
                in0=xv[:, :, 0:3, :],
                in1=pl(_DT + 1, 2).unsqueeze(2).broadcast_to((P, 2, 3, F)),
                op=MUL,
            )
            v.reciprocal_approx_fast(out=R32(1, 1), in_=R32(0, 1))
            s.copy(out=R(_RDET, 1), in_=R32(1, 1))
            v.tensor_tensor(  # spn = x h6n + y h7n
                out=R(_SP, 3), in0=R(_M12, 3), in1=R(_M12 + 3, 3), op=ADD,
            )
            v.tensor_tensor(  # sp = spn * rdet
                out=pl(_W, 3), in0=pl(_SP, 3), in1=bc1(_RDET, 3), op=MUL,
            )
            v.tensor_scalar_add(out=R(_W, 3), in0=R(_W, 3), scalar1=1.0)
            # XW = w*X, YW = w*Y (p=0..2)
            v.tensor_tensor(
                out=ab(_XW, 6, 2, 3),
                in0=pl(_W, 3).unsqueeze(1).broadcast_to((P, 2, 3, F)),
                in1=ab(_U, 8, 2, 4)[:, :, 0:3, :],
                op=MUL,
            )
            # h6 h7 -> out planes 6,7 (off the critical path)
            v.tensor_tensor(
                out=pl(_OUT + 6, 2),
                in0=pl(_DT + 1, 2),
                in1=bc1(_RDET, 2),
                op=MUL,
            )
            nc.sync.dma_start(
                out=out[6:8, :].rearrange("k (p f) -> p k f", p=P),
                in_=pl(_OUT + 6, 2),
            )
            # P1 P2 Q1 Q2
            v.tensor_tensor(
                out=ab(_PQ, 4, 2, 2),
                in0=ab(_XW, 6, 2, 3)[:, :, 1:3, :],
                in1=ab(_XW, 6, 2, 3)[:, :, 0:1, :].broadcast_to((P, 2, 2, F)),
                op=SUB,
            )
            # pe = (dy2*P1, dy2*Q1, dx1*P2, dx1*Q2), pf likewise: one fused
            # instruction each via signed-stride views
            v.tensor_tensor(
                out=ab(_PE, 4, 2, 2),
                in0=sv(_PQ, 1, 2, 2, 2),       # P1 Q1 / P2 Q2
                in1=sv(DY2, DX1 - DY2, 2, 0, 2),
                op=MUL,
            )
            v.tensor_tensor(
                out=ab(_PF, 4, 2, 2),
                in0=sv(_PQ + 1, -1, 2, 2, 2),  # P2 Q2 / P1 Q1
                in1=sv(DY1, DX2 - DY1, 2, 0, 2),
                op=MUL,
            )
            v.tensor_tensor(out=R(_HN, 4), in0=R(_PE, 4), in1=R(_PF, 4), op=SUB)
            # h0 h3 h1 h4 -> out planes 0..3
            v.tensor_tensor(out=pl(_OUT, 4), in0=pl(_HN, 4), in1=bc1(_RD, 4), op=MUL)
            nc.sync.dma_start(
                out=out[0:4, :].rearrange("k (p f) -> p k f", p=P),
                in_=pl(_OUT, 4),
            )

            # h2 = XW0 - x0 h0 - y0 h1 ; h5 = YW0 - x0 h3 - y0 h4
            v.tensor_tensor(
                out=ab(_EE, 4, 2, 2),
                in0=xv[:, :, 0:1, :].broadcast_to((P, 2, 2, F)),
                in1=ab(_OUT, 4, 2, 2),
                op=MUL,
            )  # (x0h0, x0h3, y0h1, y0h4)
            v.tensor_tensor(
                out=pl(_S1P, 2),
                in0=ab(_XW, 6, 2, 3)[:, :, 0, :],
                in1=pl(_EE, 2),
                op=SUB,
            )
            v.tensor_tensor(
                out=pl(_OUT + 4, 2), in0=pl(_S1P, 2), in1=pl(_EE + 2, 2), op=SUB
            )
            s.dma_start(
                out=out[4:6, :].rearrange("k (p f) -> p k f", p=P),
                in_=pl(_OUT + 4, 2),
            )
    nc.finalize()
    return nc


_NC_CACHE = {}


def _get_nc():
    if "nc" not in _NC_CACHE:
        _NC_CACHE["nc"] = _build()
    return _NC_CACHE["nc"]


def kernel(pts_1_tile, pred_h4p_tile, _trace=False):
    pts = np.asarray(pts_1_tile, dtype=np.float32).reshape(B_TOTAL, 8)
    prd = np.asarray(pred_h4p_tile, dtype=np.float32).reshape(B_TOTAL, 8)
    u = pts + prd
    nc = _get_nc()
    in_maps = []
    for c in range(N_CORES):
        lo, hi = c * PER_CORE, (c + 1) * PER_CORE
        xy = np.ascontiguousarray(pts[lo:hi, _PERM].T.astype(np.float16))
        uu = np.ascontiguousarray(u[lo:hi, _PERM].T.astype(np.float16))
        in_maps.append({"xy": xy, "uu": uu, "eye": _EYE})
    res = run_bass_kernel_spmd(nc, in_maps, list(range(N_CORES)), trace=_trace)
    arr = np.stack([res.results[i]["out"] for i in range(N_CORES)], axis=0)
    H9 = np.empty((N_CORES, PER_CORE, 9), np.float32)
    for k, m in enumerate(_PLANE2H):
        H9[:, :, m] = arr[:, k, :]
    H9[:, :, 8] = 1.0
    H = H9.reshape(B_TOTAL, 3, 3)
    if _trace:
        return H, res
    return H
